# revision 9
# baseline (speedup 1.0000x reference)
# Trainium2 Bass kernel for EuclideanCodebook (VQ) — data-parallel over tokens.
#
# reference semantics (fp32):
#   f = x            [N, D]
#   e = embed[0]     [K, D]
#   dist[n,k] = max(x_sq[n] + e_sq[k] - 2 * f @ e.T, 0)    [N, K]
#   embed_ind = argmin_k dist                               [N]
#   quantize = e[embed_ind]                                 [N, D]
# returns (quantize, embed_ind[None], dist[None])
#
# Sharding: x split into 8 shards of N/8 tokens (one per NeuronCore); the
# codebook is replicated. Each core computes its [N/8, K] distance block,
# argmin, and gather locally; host concatenates shard outputs.
#
# Numerics: the N x K matmul runs as 3 bf16 passes (hi*hi, hi*lo, lo*hi of
# the bf16 split of -2x and e) accumulating in fp32 PSUM, which matches
# fp32 argmin exactly on this input (verified: 0 flips vs the reference,
# min top-2 margin 2.1e-4 >> accumulation noise). e_sq enters the same
# accumulation via an augmented c=32 matmul whose rhs rows are a 3-term
# bf16 split of e_sq (exact to ~2^-24 rel); x_sq is added per-partition by
# the ScalarE bias during the PSUM->SBUF relu copy (no argmin effect).

import numpy as np

N_FULL = 32768
D = 256
K = 4096
N_CORES = 8
NSH = N_FULL // N_CORES  # 4096 tokens per core
P = 128
NT = NSH // P            # 32 token tiles per core
KT = K // P              # 32 codebook tiles
DT = D // P              # 2 contraction subtiles
KH = K // 2              # 2048, k-half per PSUM group
MMF = 512                # matmul free dim (one PSUM bank of fp32)

_COMPILED = {}


def _build_program():
    import concourse.bacc as bacc
    import concourse.bass as bass
    import concourse.mybir as mybir
    import concourse.tile as tile
    from concourse.bass import ts
    from concourse.masks import make_identity

    f32 = mybir.dt.float32
    bf16 = mybir.dt.bfloat16
    u32 = mybir.dt.uint32
    i32 = mybir.dt.int32
    AF = mybir.ActivationFunctionType
    ALU = mybir.AluOpType
    AX = mybir.AxisListType

    nc = bacc.Bacc("TRN2", target_bir_lowering=False, debug=False,
                   enable_asserts=False, num_devices=N_CORES)

    xs = nc.dram_tensor("xs", [NSH, D], f32, kind="ExternalInput").ap()
    emb = nc.dram_tensor("emb", [K, D], f32, kind="ExternalInput").ap()
    dist_o = nc.dram_tensor("dist_o", [NSH, K], f32, kind="ExternalOutput").ap()
    quant_o = nc.dram_tensor("quant_o", [NSH, D], f32, kind="ExternalOutput").ap()
    ind_o = nc.dram_tensor("ind_o", [NSH], i32, kind="ExternalOutput").ap()

    with tile.TileContext(nc) as tc:
        import contextlib
        with contextlib.ExitStack() as ctx:
            const = ctx.enter_context(tc.tile_pool(name="const", bufs=1))
            prep = ctx.enter_context(tc.tile_pool(name="prep", bufs=2))
            dram = ctx.enter_context(tc.tile_pool(name="dram", bufs=1, space="DRAM"))
            psum = ctx.enter_context(tc.tile_pool(name="psum", bufs=2, space="PSUM"))
            work = ctx.enter_context(tc.tile_pool(name="work", bufs=3))
            small = ctx.enter_context(tc.tile_pool(name="small", bufs=1))

            # ---- persistent operands ----
            xTh = const.tile([P, DT, NSH], bf16)  # transposed bf16-hi of -2x
            xTl = const.tile([P, DT, NSH], bf16)  # transposed bf16-lo of -2x
            eTh = const.tile([P, DT, K], bf16)
            eTl = const.tile([P, DT, K], bf16)
            xsq = const.tile([P, NT], f32)        # x_sq, col t = token tile t
            esq_rows = const.tile([32, K], bf16)  # rows 0..2: bf16 split of e_sq
            ones_l = const.tile([32, P], bf16)    # aug stationary: rows 0..2 ones
            nc.vector.memset(ones_l[:], 0.0)
            nc.vector.memset(ones_l[0:3, :], 1.0)
            nc.vector.memset(esq_rows[:], 0.0)

            identity = const.tile([P, P], f32)
            make_identity(nc, identity[:])

            # DRAM scratch for the bf16 hi/lo planes (xbar-transposed back in)
            xh_d = dram.tile([NSH, D], bf16)
            xl_d = dram.tile([NSH, D], bf16)
            eh_d = dram.tile([K, D], bf16)
            el_d = dram.tile([K, D], bf16)
            esq_d = dram.tile([3, KT, P], bf16)

            esq_sb = const.tile([P, KT], f32)

            # ---- prep: squares, hi/lo bf16 splits, chunked (1024 rows) ----
            # The -2 of "-2 x.e" is folded into the e-side planes: eh =
            # bf16(-2e), el = bf16(-2e - eh). x planes are a plain bf16 split.
            CH = 1024            # rows per chunk
            CT = CH // P         # 8 token/code tiles per chunk
            def split_chunk(src_ap, c, hi_dram, lo_dram, sq_col, fold_m2, Th, Tl):
                ld = prep.tile([P, CT, D], f32, tag="ld")
                nc.sync.dma_start(
                    ld[:], src_ap[c * CH:(c + 1) * CH, :]
                    .rearrange("(a p) d -> p a d", p=P))
                for a in range(CT):
                    sq_scr = prep.tile([P, D], f32, tag="sq", bufs=2)
                    nc.scalar.activation(
                        sq_scr[:], ld[:, a, :], AF.Square,
                        accum_out=sq_col[:, c * CT + a:c * CT + a + 1])
                hi = prep.tile([P, CT, D], bf16, tag="hi")
                lo = prep.tile([P, CT, D], bf16, tag="lo")
                if fold_m2:
                    nc.vector.tensor_scalar_mul(hi[:], ld[:], -2.0)
                    nc.vector.scalar_tensor_tensor(
                        lo[:], ld[:], -2.0, hi[:], op0=ALU.mult, op1=ALU.subtract)
                else:
                    nc.vector.tensor_copy(hi[:], ld[:])
                    nc.vector.scalar_tensor_tensor(
                        lo[:], hi[:], -1.0, ld[:], op0=ALU.mult, op1=ALU.add)
                nc.sync.dma_start(
                    hi_dram[c * CH:(c + 1) * CH, :]
                    .rearrange("(a p) d -> p a d", p=P), hi[:])
                nc.sync.dma_start(
                    lo_dram[c * CH:(c + 1) * CH, :]
                    .rearrange("(a p) d -> p a d", p=P), lo[:])
                for j in range(DT):
                    nc.sync.dma_start_transpose(
                        Th[:, j, c * CH:(c + 1) * CH],
                        hi_dram[c * CH:(c + 1) * CH, ts(j, P)])
                    nc.sync.dma_start_transpose(
                        Tl[:, j, c * CH:(c + 1) * CH],
                        lo_dram[c * CH:(c + 1) * CH, ts(j, P)])

            # e_sq 3-term bf16 split, per chunk: cols -> [24, 128] PE transpose
            # -> bf16 staging in DRAM (k-ordered) -> one DMA into esq_rows.
            def esq_chunk(c):
                cols = slice(c * CT, (c + 1) * CT)
                parts = small.tile([P, 3, CT], f32, tag="parts", bufs=2)
                e1 = small.tile([P, CT], bf16, tag="e1", bufs=2)
                nc.vector.tensor_copy(e1[:], esq_sb[:, cols])
                r1 = small.tile([P, CT], f32, tag="r1", bufs=2)
                nc.vector.scalar_tensor_tensor(
                    r1[:], e1[:], -1.0, esq_sb[:, cols], op0=ALU.mult, op1=ALU.add)
                e2 = small.tile([P, CT], bf16, tag="e2", bufs=2)
                nc.vector.tensor_copy(e2[:], r1[:])
                r2 = small.tile([P, CT], f32, tag="r2", bufs=2)
                nc.vector.scalar_tensor_tensor(
                    r2[:], e2[:], -1.0, r1[:], op0=ALU.mult, op1=ALU.add)
                nc.vector.tensor_copy(parts[:, 0, :], e1[:])
                nc.vector.tensor_copy(parts[:, 1, :], e2[:])
                nc.vector.tensor_copy(parts[:, 2, :], r2[:])
                ps_slot = psum.tile([P, KH], f32, tag="ps")
                ps24 = ps_slot[0:3 * CT, 0:P]
                nc.tensor.transpose(
                    ps24, parts[:].rearrange("p a b -> p (a b)"), identity[:])
                sb24 = small.tile([3 * CT, P], bf16, tag="sb24", bufs=2)
                nc.scalar.copy(sb24[:], ps24)
                for r in range(3):
                    nc.gpsimd.dma_start(
                        esq_d[r, cols, :], sb24[r * CT:(r + 1) * CT, :])

            for c in range(K // CH):
                split_chunk(emb, c, eh_d[:], el_d[:], esq_sb, True, eTh, eTl)
                esq_chunk(c)
            nc.gpsimd.dma_start(
                esq_rows[0:3, :], esq_d[:].rearrange("a b p -> a (b p)"))

            for c in range(NSH // CH):
                split_chunk(xs, c, xh_d[:], xl_d[:], xsq, False, xTh, xTl)

            # ---- main loop over token tiles ----
            passes = ((xTh, eTh), (xTh, eTl), (xTl, eTh))
            for t in range(NT):
                dist_sb = work.tile([P, K], f32, tag="dist")
                for h in range(2):
                    ps = psum.tile([P, KH], f32, tag="ps")
                    for p_i, (xw, ew) in enumerate(passes):
                        for j in range(DT):
                            lhsT = xw[:, j, ts(t, P)]
                            first = (p_i == 0 and j == 0)
                            for c in range(KH // MMF):
                                nc.tensor.matmul(
                                    ps[:, ts(c, MMF)], lhsT,
                                    ew[:, j, h * KH + c * MMF:
                                       h * KH + (c + 1) * MMF],
                                    start=first, stop=False)
                    for c in range(KH // MMF):
                        nc.tensor.matmul(
                            ps[:, ts(c, MMF)], ones_l[:],
                            esq_rows[:, h * KH + c * MMF:h * KH + (c + 1) * MMF],
                            start=False, stop=True)
                    nc.scalar.activation(dist_sb[:, h * KH:(h + 1) * KH], ps[:],
                                         AF.Relu, bias=xsq[:, t:t + 1], scale=1.0)
                nc.sync.dma_start(dist_o[ts(t, P), :], dist_sb[:])

                m_col = work.tile([P, 1], f32, tag="mcol")
                nc.vector.tensor_reduce(m_col[:], dist_sb[:], axis=AX.X, op=ALU.min)
                m8 = work.tile([P, 8], f32, tag="m8")
                nc.scalar.copy(m8[:], m_col[:, 0:1].to_broadcast((P, 8)))
                idx8 = work.tile([P, 8], u32, tag="idx8")
                nc.vector.max_index(idx8[:], m8[:], dist_sb[:])
                idx_i = work.tile([P, 1], i32, tag="idxi")
                nc.gpsimd.tensor_copy(idx_i[:], idx8[:, 0:1])
                nc.sync.dma_start(ind_o[ts(t, P), None], idx_i[:])

                q_t = work.tile([P, D], f32, tag="q")
                nc.gpsimd.indirect_dma_start(
                    out=q_t[:], out_offset=None, in_=emb[:, :],
                    in_offset=bass.IndirectOffsetOnAxis(ap=idx_i[:, 0:1], axis=0))
                nc.sync.dma_start(quant_o[ts(t, P), :], q_t[:])

    nc.compile()
    return nc


def _get_program():
    if "nc" not in _COMPILED:
        _COMPILED["nc"] = _build_program()
    return _COMPILED["nc"]


def kernel(x: np.ndarray, embed: np.ndarray):
    from concourse.bass_utils import run_bass_kernel_spmd

    nc = _get_program()
    x = np.ascontiguousarray(np.asarray(x, dtype=np.float32))
    e = np.ascontiguousarray(np.asarray(embed, dtype=np.float32)[0])
    in_maps = [
        {"xs": x[i * NSH:(i + 1) * NSH], "emb": e} for i in range(N_CORES)
    ]
    res = run_bass_kernel_spmd(nc, in_maps, core_ids=list(range(N_CORES)))
    outs = res.results
    quant = np.concatenate([o["quant_o"] for o in outs], axis=0)
    ind = np.concatenate([o["ind_o"] for o in outs], axis=0).astype(np.int32)[None]
    dist = np.concatenate([o["dist_o"] for o in outs], axis=0)[None]
    return quant, ind, dist


# revision 10
# speedup vs baseline: 1.1709x; 1.1709x over previous
# Trainium2 Bass kernel for EuclideanCodebook (VQ) — data-parallel over tokens.
#
# reference semantics (fp32):
#   f = x            [N, D]
#   e = embed[0]     [K, D]
#   dist[n,k] = max(x_sq[n] + e_sq[k] - 2 * f @ e.T, 0)    [N, K]
#   embed_ind = argmin_k dist                               [N]
#   quantize = e[embed_ind]                                 [N, D]
# returns (quantize, embed_ind[None], dist[None])
#
# Sharding: x split into 8 shards of N/8 tokens (one per NeuronCore); the
# codebook is replicated. Each core computes its [N/8, K] distance block,
# argmin, and gather locally; host concatenates shard outputs.
#
# Numerics: the N x K matmul runs as 3 bf16 passes (hi*hi, hi*lo, lo*hi of
# the bf16 split of -2x and e) accumulating in fp32 PSUM, which matches
# fp32 argmin exactly on this input (verified: 0 flips vs the reference,
# min top-2 margin 2.1e-4 >> accumulation noise). e_sq enters the same
# accumulation via an augmented c=32 matmul whose rhs rows are a 3-term
# bf16 split of e_sq (exact to ~2^-24 rel); x_sq is added per-partition by
# the ScalarE bias during the PSUM->SBUF relu copy (no argmin effect).

import numpy as np

N_FULL = 32768
D = 256
K = 4096
N_CORES = 8
NSH = N_FULL // N_CORES  # 4096 tokens per core
P = 128
NT = NSH // P            # 32 token tiles per core
KT = K // P              # 32 codebook tiles
DT = D // P              # 2 contraction subtiles
KH = K // 2              # 2048, k-half per PSUM group
MMF = 512                # matmul free dim (one PSUM bank of fp32)

_COMPILED = {}


def _build_program():
    import concourse.bacc as bacc
    import concourse.bass as bass
    import concourse.mybir as mybir
    import concourse.tile as tile
    from concourse.bass import ts
    from concourse.masks import make_identity

    f32 = mybir.dt.float32
    bf16 = mybir.dt.bfloat16
    u32 = mybir.dt.uint32
    i32 = mybir.dt.int32
    AF = mybir.ActivationFunctionType
    ALU = mybir.AluOpType
    AX = mybir.AxisListType

    nc = bacc.Bacc("TRN2", target_bir_lowering=False, debug=False,
                   enable_asserts=False, num_devices=N_CORES)

    xs = nc.dram_tensor("xs", [NSH, D], f32, kind="ExternalInput").ap()
    emb = nc.dram_tensor("emb", [K, D], f32, kind="ExternalInput").ap()
    dist_o = nc.dram_tensor("dist_o", [NSH, K], f32, kind="ExternalOutput").ap()
    quant_o = nc.dram_tensor("quant_o", [NSH, D], f32, kind="ExternalOutput").ap()
    ind_o = nc.dram_tensor("ind_o", [NSH], i32, kind="ExternalOutput").ap()

    with tile.TileContext(nc) as tc:
        import contextlib
        with contextlib.ExitStack() as ctx:
            const = ctx.enter_context(tc.tile_pool(name="const", bufs=1))
            prep = ctx.enter_context(tc.tile_pool(name="prep", bufs=2))
            dram = ctx.enter_context(tc.tile_pool(name="dram", bufs=1, space="DRAM"))
            psum = ctx.enter_context(tc.tile_pool(name="psum", bufs=2, space="PSUM"))
            work = ctx.enter_context(tc.tile_pool(name="work", bufs=3))
            small = ctx.enter_context(tc.tile_pool(name="small", bufs=1))

            # ---- persistent operands ----
            xTh = const.tile([P, DT, NSH], bf16)  # transposed bf16-hi of -2x
            xTl = const.tile([P, DT, NSH], bf16)  # transposed bf16-lo of -2x
            eTh = const.tile([P, DT, K], bf16)
            eTl = const.tile([P, DT, K], bf16)
            xsq = const.tile([P, NT], f32)        # x_sq, col t = token tile t
            esq_rows = const.tile([32, K], bf16)  # rows 0..2: bf16 split of e_sq
            ones_l = const.tile([32, P], bf16)    # aug stationary: rows 0..2 ones
            nc.vector.memset(ones_l[:], 0.0)
            nc.vector.memset(ones_l[0:3, :], 1.0)
            nc.vector.memset(esq_rows[:], 0.0)

            identity = const.tile([P, P], f32)
            make_identity(nc, identity[:])

            # DRAM scratch for the bf16 hi/lo planes (xbar-transposed back in)
            xh_d = dram.tile([NSH, D], bf16)
            xl_d = dram.tile([NSH, D], bf16)
            eh_d = dram.tile([K, D], bf16)
            el_d = dram.tile([K, D], bf16)
            esq_d = dram.tile([3, KT, P], bf16)

            esq_sb = const.tile([P, KT], f32)

            # ---- prep: squares, hi/lo bf16 splits, chunked (1024 rows) ----
            # The -2 of "-2 x.e" is folded into the e-side planes: eh =
            # bf16(-2e), el = bf16(-2e - eh). x planes are a plain bf16 split.
            CH = 1024            # rows per chunk
            CT = CH // P         # 8 token/code tiles per chunk
            def split_chunk(src_ap, c, hi_dram, lo_dram, sq_col, fold_m2, Th, Tl):
                ld = prep.tile([P, CT, D], f32, tag="ld")
                nc.sync.dma_start(
                    ld[:], src_ap[c * CH:(c + 1) * CH, :]
                    .rearrange("(a p) d -> p a d", p=P))
                for a in range(CT):
                    sq_scr = prep.tile([P, D], f32, tag="sq", bufs=2)
                    nc.scalar.activation(
                        sq_scr[:], ld[:, a, :], AF.Square,
                        accum_out=sq_col[:, c * CT + a:c * CT + a + 1])
                hi = prep.tile([P, CT, D], bf16, tag="hi")
                lo = prep.tile([P, CT, D], bf16, tag="lo")
                if fold_m2:
                    nc.vector.tensor_scalar_mul(hi[:], ld[:], -2.0)
                    nc.vector.scalar_tensor_tensor(
                        lo[:], ld[:], -2.0, hi[:], op0=ALU.mult, op1=ALU.subtract)
                else:
                    nc.vector.tensor_copy(hi[:], ld[:])
                    nc.vector.scalar_tensor_tensor(
                        lo[:], hi[:], -1.0, ld[:], op0=ALU.mult, op1=ALU.add)
                nc.sync.dma_start(
                    hi_dram[c * CH:(c + 1) * CH, :]
                    .rearrange("(a p) d -> p a d", p=P), hi[:])
                nc.sync.dma_start(
                    lo_dram[c * CH:(c + 1) * CH, :]
                    .rearrange("(a p) d -> p a d", p=P), lo[:])
                for j in range(DT):
                    nc.sync.dma_start_transpose(
                        Th[:, j, c * CH:(c + 1) * CH],
                        hi_dram[c * CH:(c + 1) * CH, ts(j, P)])
                    nc.sync.dma_start_transpose(
                        Tl[:, j, c * CH:(c + 1) * CH],
                        lo_dram[c * CH:(c + 1) * CH, ts(j, P)])

            # e chunks first (full eT needed for any tile), then x chunk 0
            # (the only x dependency of tile 0), then e_sq, then the rest of x.
            for c in range(K // CH):
                split_chunk(emb, c, eh_d[:], el_d[:], esq_sb, True, eTh, eTl)
            split_chunk(xs, 0, xh_d[:], xl_d[:], xsq, False, xTh, xTl)

            # ---- e_sq 3-term bf16 split -> [3, K] rows (PE-transpose + DRAM) ----
            parts96 = small.tile([P, 3, KT], f32)
            e1 = small.tile([P, KT], bf16)
            nc.vector.tensor_copy(e1[:], esq_sb[:])
            r1 = small.tile([P, KT], f32)
            nc.vector.scalar_tensor_tensor(
                r1[:], e1[:], -1.0, esq_sb[:], op0=ALU.mult, op1=ALU.add)
            e2 = small.tile([P, KT], bf16)
            nc.vector.tensor_copy(e2[:], r1[:])
            r2 = small.tile([P, KT], f32)
            nc.vector.scalar_tensor_tensor(
                r2[:], e2[:], -1.0, r1[:], op0=ALU.mult, op1=ALU.add)
            nc.vector.tensor_copy(parts96[:, 0, :], e1[:])
            nc.vector.tensor_copy(parts96[:, 1, :], e2[:])
            nc.vector.tensor_copy(parts96[:, 2, :], r2[:])
            # transpose [128, 96] -> [96, 128]: partition r*KT+t holds e_sq
            # part r of code tile t -> flat DRAM [3, KT, 128] is k-ordered.
            # The ACT copy rounds part 3 (r2) to bf16, completing the split.
            ps_slot = psum.tile([P, KH], f32, tag="ps")
            ps96 = ps_slot[0:96, 0:P]
            nc.tensor.transpose(ps96, parts96[:].rearrange("p a b -> p (a b)"),
                                identity[:])
            sb96 = small.tile([96, P], bf16)
            nc.scalar.copy(sb96[:], ps96)
            nc.scalar.dma_start(
                esq_d[:].rearrange("a b p -> (a b) p"), sb96[:])
            nc.scalar.dma_start(
                esq_rows[0:3, :], esq_d[:].rearrange("a b p -> a (b p)"))

            for c in range(1, NSH // CH):
                split_chunk(xs, c, xh_d[:], xl_d[:], xsq, False, xTh, xTl)

            # ---- main loop over token tiles ----
            passes = ((xTh, eTh), (xTh, eTl), (xTl, eTh))
            for t in range(NT):
                dist_sb = work.tile([P, K], f32, tag="dist")
                for h in range(2):
                    ps = psum.tile([P, KH], f32, tag="ps")
                    for p_i, (xw, ew) in enumerate(passes):
                        for j in range(DT):
                            lhsT = xw[:, j, ts(t, P)]
                            first = (p_i == 0 and j == 0)
                            for c in range(KH // MMF):
                                nc.tensor.matmul(
                                    ps[:, ts(c, MMF)], lhsT,
                                    ew[:, j, h * KH + c * MMF:
                                       h * KH + (c + 1) * MMF],
                                    start=first, stop=False)
                    for c in range(KH // MMF):
                        nc.tensor.matmul(
                            ps[:, ts(c, MMF)], ones_l[:],
                            esq_rows[:, h * KH + c * MMF:h * KH + (c + 1) * MMF],
                            start=False, stop=True)
                    nc.scalar.activation(dist_sb[:, h * KH:(h + 1) * KH], ps[:],
                                         AF.Relu, bias=xsq[:, t:t + 1], scale=1.0)
                nc.sync.dma_start(dist_o[ts(t, P), :], dist_sb[:])

                m_col = work.tile([P, 1], f32, tag="mcol")
                nc.vector.tensor_reduce(m_col[:], dist_sb[:], axis=AX.X, op=ALU.min)
                m8 = work.tile([P, 8], f32, tag="m8")
                nc.scalar.copy(m8[:], m_col[:, 0:1].to_broadcast((P, 8)))
                idx8 = work.tile([P, 8], u32, tag="idx8")
                nc.vector.max_index(idx8[:], m8[:], dist_sb[:])
                idx_i = work.tile([P, 1], i32, tag="idxi")
                nc.gpsimd.tensor_copy(idx_i[:], idx8[:, 0:1])
                nc.sync.dma_start(ind_o[ts(t, P), None], idx_i[:])

                q_t = work.tile([P, D], f32, tag="q")
                nc.gpsimd.indirect_dma_start(
                    out=q_t[:], out_offset=None, in_=emb[:, :],
                    in_offset=bass.IndirectOffsetOnAxis(ap=idx_i[:, 0:1], axis=0))
                nc.sync.dma_start(quant_o[ts(t, P), :], q_t[:])

    nc.compile()
    return nc


def _get_program():
    if "nc" not in _COMPILED:
        _COMPILED["nc"] = _build_program()
    return _COMPILED["nc"]


def kernel(x: np.ndarray, embed: np.ndarray):
    from concourse.bass_utils import run_bass_kernel_spmd

    nc = _get_program()
    x = np.ascontiguousarray(np.asarray(x, dtype=np.float32))
    e = np.ascontiguousarray(np.asarray(embed, dtype=np.float32)[0])
    in_maps = [
        {"xs": x[i * NSH:(i + 1) * NSH], "emb": e} for i in range(N_CORES)
    ]
    res = run_bass_kernel_spmd(nc, in_maps, core_ids=list(range(N_CORES)))
    outs = res.results
    quant = np.concatenate([o["quant_o"] for o in outs], axis=0)
    ind = np.concatenate([o["ind_o"] for o in outs], axis=0).astype(np.int32)[None]
    dist = np.concatenate([o["dist_o"] for o in outs], axis=0)[None]
    return quant, ind, dist
